# revision 1
# baseline (speedup 1.0000x reference)
"""Trainium2 Bass kernel v2 for nn_AR_decoder (autoregressive LSTM decoder).

Contract: kernel(**inputs) takes FULL unsharded numpy inputs, returns FULL
output [256, 2048, 5] f32 (per-step log_softmax of the decoder).

Design (per core, 32 batch rows, T=2048 sequential steps):
  - 2 phase-shifted streams of 16 batch rows; the streams' serial chains
    overlap across engines.
  - x @ Wx.T precomputed by bulk matmuls (16 steps at a time) directly into
    PSUM; the per-step matmul accumulates Whh@h + Wpe@onehot on top
    (start=False into a column slice).
  - Gate order [i,f,o,g]; ACT ops per step: sigmoid(rows 0:96), tanh(g) with
    base-shifted output, tanh(c). Cell math on DVE with partition-base-aligned
    operand pairs (verifier requires equal input bases for TensorTensor).
  - Argmax feedback: fc matmul (h stationary) -> l[16,5] psum; DVE copy->ls,
    reduce_max, is_equal, 32x32 transpose into the moving tile's onehot rows.
    Stream-interleaved DVE tail provides the hazard distance (no drains).
  - mov tile rows: 0=ones(tr col0), 1-5=onehot, 6-63=0, 64-95=h, 96=ones(fc
    bias row). Gates contraction = rows 0:97, fc contraction = rows 64:97.
  - Raw logits DMA'd out; log_softmax on host.
"""

import os
import numpy as np
from contextlib import ExitStack

import concourse.bass as bass
from concourse import mybir
from concourse.bass_utils import run_bass_kernel_spmd

B, T, IN, H, NCLS = 256, 2048, 64, 32, 5
NCORES = 8
BL = B // NCORES          # 32 batch rows per core
NS = 2                    # streams per core
BS = BL // NS             # 16 batch rows per stream

AF = mybir.ActivationFunctionType
ALU = mybir.AluOpType
AX = mybir.AxisListType
F32 = mybir.dt.float32

_TT = int(os.environ.get("KB_TT", T))
_SKIP = set(filter(None, os.environ.get("KB_SKIP", "").split(",")))
_TRACE = os.environ.get("KB_TRACE", "0") == "1"

LAST_EXEC_NS = None
LAST_RESULTS = None


def _sched(TT, XWC):
    """Dry-run the emission order, assigning 1-based semaphore ticks."""
    tick = {}
    cnt = {"pe": 0, "act": 0, "dve": 0}

    def put(eng, key):
        cnt[eng] += 1
        tick[(eng,) + key] = cnt[eng]

    for s in range(NS):
        for k in range(7):
            put("dve", ("init", s, k))
    for t in range(TT):
        for s in range(NS):
            if t % XWC == 0:
                put("pe", ("bulk", t, s))
        for s in range(NS):
            put("pe", ("gates", t, s))
        if "fc" not in _SKIP:
            for s in range(NS):
                put("pe", ("fc", t, s))
        for s in range(NS):
            put("act", ("sig", t, s))
            put("act", ("tang", t, s))
        for s in range(NS):
            put("act", ("tanc", t, s))
        for s in range(NS):
            put("dve", ("mul1", t, s))
            put("dve", ("mul2", t, s))
            put("dve", ("add", t, s))
        for s in range(NS):
            put("dve", ("h", t, s))
        if "fc" not in _SKIP:
            for s in range(NS):
                put("dve", ("copy", t, s))
                put("dve", ("max", t, s))
            for s in range(NS):
                put("dve", ("eq", t, s))
        if "tr" not in _SKIP:
            for s in range(NS):
                put("dve", ("tr", t, s))
    return tick


def _build(TT):
    XWC = min(16, TT)          # steps per bulk xw matmul
    XC = min(128, TT)          # steps per x DMA chunk
    OC = min(256, TT)          # steps per output DMA chunk
    assert TT % XWC == 0 and TT % XC == 0 and TT % OC == 0

    tk = _sched(TT, XWC)
    nc = bass.Bass()

    x_d = [nc.declare_dram_parameter(f"xT{s}", [IN, TT * BS], F32,
                                     isOutput=False) for s in range(NS)]
    ws_d = nc.declare_dram_parameter("wstack", [97, 128], F32, isOutput=False)
    wf_d = nc.declare_dram_parameter("wfc", [97, NCLS], F32, isOutput=False)
    wx_d = nc.declare_dram_parameter("wxT", [IN, 128], F32, isOutput=False)
    b_d = nc.declare_dram_parameter("ball", [128, 1], F32, isOutput=False)
    out_d = nc.declare_dram_parameter("out", [BL, TT * NCLS], F32,
                                      isOutput=True)

    with ExitStack() as ctx:
        def sb(name, shape):
            return ctx.enter_context(nc.sbuf_tensor(name, shape, F32))

        xr = [[sb(f"xr{s}_{p}", [IN, XC * BS]) for p in range(2)]
              for s in range(NS)]
        mov = [[sb(f"mov{s}_{p}", [97, 32]) for p in range(2)]
               for s in range(NS)]
        S = [sb(f"S{s}", [96, BS]) for s in range(NS)]
        Q = [sb(f"Q{s}", [32, BS]) for s in range(NS)]       # tanh(g)
        C = [sb(f"C{s}", [64, BS]) for s in range(NS)]       # c at rows 32:64
        P = [sb(f"P{s}", [32, BS]) for s in range(NS)]       # i*tanh(g)
        P2 = [sb(f"P2{s}", [32, BS]) for s in range(NS)]     # f*c
        tco = [sb(f"tco{s}", [96, BS]) for s in range(NS)]   # tanh(c) @64:96
        ohs = [sb(f"ohs{s}", [32, 32]) for s in range(NS)]
        ms = [sb(f"ms{s}", [BS, 1]) for s in range(NS)]
        lsb = [sb(f"ls{s}", [BS, TT * NCLS]) for s in range(NS)]
        wstack = sb("wstack_sb", [97, 128])
        wfc = sb("wfc_sb", [97, NCLS])
        wxT = sb("wxT_sb", [IN, 128])
        ball = sb("ball_sb", [128, 1])

        xwg = [[ctx.enter_context(
            nc.psum_tensor(f"xwg{s}_{p}", [128, XWC * BS], F32))
            for p in range(2)] for s in range(NS)]
        l5 = [[ctx.enter_context(
            nc.psum_tensor(f"l5{s}_{p}", [BS, NCLS], F32))
            for p in range(2)] for s in range(NS)]

        s_dmax = [ctx.enter_context(nc.semaphore(f"s_dmax{s}"))
                  for s in range(NS)]
        s_dmaw = ctx.enter_context(nc.semaphore("s_dmaw"))
        s_dmao = ctx.enter_context(nc.semaphore("s_dmao"))
        s_pe = ctx.enter_context(nc.semaphore("s_pe"))
        s_act = ctx.enter_context(nc.semaphore("s_act"))
        s_dve = ctx.enter_context(nc.semaphore("s_dve"))

        DVE_INIT = 7 * NS

        with nc.Block() as block:

            @block.sync
            def _(sync):
                for dst, src in ((wstack, ws_d), (wfc, wf_d), (wxT, wx_d),
                                 (ball, b_d)):
                    sync.dma_start(dst[:], src[:]).then_inc(s_dmaw, 16)
                for k in range(TT // XC):
                    for s in range(NS):
                        if k >= 2:
                            tlast = (k - 1) * XC - XWC
                            sync.wait_ge(s_pe, tk[("pe", "bulk", tlast, s)])
                        sync.dma_start(
                            xr[s][k % 2][:],
                            x_d[s][:, k * XC * BS:(k + 1) * XC * BS],
                        ).then_inc(s_dmax[s], 16)
                for j in range(TT // OC if "fc" not in _SKIP else 0):
                    tlast = (j + 1) * OC - 1
                    for s in range(NS):
                        sync.wait_ge(s_dve, tk[("dve", "copy", tlast, s)])
                        sync.dma_start(
                            out_d[s * BS:(s + 1) * BS,
                                  j * OC * NCLS:(j + 1) * OC * NCLS],
                            lsb[s][:, j * OC * NCLS:(j + 1) * OC * NCLS],
                        ).then_inc(s_dmao, 16)
                if "fc" not in _SKIP:
                    sync.wait_ge(s_dmao, 16 * (TT // OC) * NS)

            @block.tensor
            def _(pe):
                pe.wait_ge(s_dmaw, 64)
                for t in range(TT):
                    par = (t // XWC) % 2
                    for s in range(NS):
                        if t % XWC == 0:
                            if t % XC == 0:
                                k = t // XC
                                pe.wait_ge(s_dmax[s], 16 * (k + 1))
                            blk = t // XWC
                            if blk >= 2:
                                tlast = (blk - 1) * XWC - 1
                                pe.wait_ge(
                                    s_act, tk[("act", "tang", tlast, s)])
                            off = (t % XC) * BS
                            nc.tensor.matmul(
                                xwg[s][par][:],
                                wxT[:],
                                xr[s][(t // XC) % 2][:, off:off + XWC * BS],
                                start=True, stop=False,
                                skip_group_check=True,
                            ).then_inc(s_pe)
                    for s in range(NS):
                        if t == 0:
                            pe.wait_ge(s_dve, DVE_INIT)
                        else:
                            lbl = ("tr" if "tr" not in _SKIP else
                                   ("eq" if "fc" not in _SKIP else "h"))
                            pe.wait_ge(s_dve, tk[("dve", lbl, t - 1, s)])
                        sl = slice((t % XWC) * BS, (t % XWC + 1) * BS)
                        nc.tensor.matmul(
                            xwg[s][par][:, sl],
                            wstack[:],
                            mov[s][t % 2][0:97, 0:BS],
                            start=False, stop=True,
                            skip_group_check=True,
                        ).then_inc(s_pe)
                    if "fc" not in _SKIP:
                        for s in range(NS):
                            pe.wait_ge(s_dve, tk[("dve", "h", t, s)])
                            nc.tensor.matmul(
                                l5[s][t % 2][:],
                                mov[s][(t + 1) % 2][64:97, 0:BS],
                                wfc[64:97, :],
                                start=True, stop=True,
                            ).then_inc(s_pe)

            @block.scalar
            def _(act):
                for t in range(TT):
                    par = (t // XWC) % 2
                    sl = slice((t % XWC) * BS, (t % XWC + 1) * BS)
                    for s in range(NS):
                        act.wait_ge(s_pe, tk[("pe", "gates", t, s)])
                        nc.scalar.activation(
                            S[s][:], xwg[s][par][0:96, sl], AF.Sigmoid,
                            bias=ball[0:96, :],
                        ).then_inc(s_act)
                        nc.scalar.activation(
                            Q[s][:], xwg[s][par][96:128, sl], AF.Tanh,
                            bias=ball[96:128, :],
                        ).then_inc(s_act)
                    for s in range(NS):
                        act.wait_ge(s_dve, tk[("dve", "add", t, s)])
                        nc.scalar.activation(
                            tco[s][64:96, :], C[s][32:64, :], AF.Tanh,
                        ).then_inc(s_act)

            @block.vector
            def _(dve):
                for s in range(NS):
                    for p in range(2):
                        nc.vector.memset(mov[s][p][:], 0.0).then_inc(s_dve)
                        nc.vector.memset(mov[s][p][96:97, :],
                                         1.0).then_inc(s_dve)
                    nc.vector.memset(C[s][32:64, :], 0.0).then_inc(s_dve)
                    nc.vector.memset(ohs[s][:], 0.0).then_inc(s_dve)
                    nc.vector.memset(ohs[s][:, 0:1], 1.0).then_inc(s_dve)
                for t in range(TT):
                    for s in range(NS):
                        dve.wait_ge(s_act, tk[("act", "tang", t, s)])
                        nc.vector.tensor_mul(
                            P[s][:], S[s][0:32, :], Q[s][:],
                        ).then_inc(s_dve)
                        nc.vector.tensor_mul(
                            P2[s][:], S[s][32:64, :], C[s][32:64, :],
                        ).then_inc(s_dve)
                        nc.vector.tensor_add(
                            C[s][32:64, :], P[s][:], P2[s][:],
                        ).then_inc(s_dve)
                    for s in range(NS):
                        dve.wait_ge(s_act, tk[("act", "tanc", t, s)])
                        nc.vector.tensor_mul(
                            mov[s][(t + 1) % 2][64:96, 0:BS], S[s][64:96, :],
                            tco[s][64:96, :],
                        ).then_inc(s_dve)
                    if "fc" not in _SKIP:
                        for s in range(NS):
                            dve.wait_ge(s_pe, tk[("pe", "fc", t, s)])
                            nc.vector.tensor_copy(
                                lsb[s][:, t * NCLS:(t + 1) * NCLS],
                                l5[s][t % 2][:],
                            ).then_inc(s_dve)
                            nc.vector.reduce_max(
                                ms[s][:], l5[s][t % 2][:], axis=AX.X,
                            ).then_inc(s_dve)
                        for s in range(NS):
                            nc.vector.tensor_scalar(
                                ohs[s][0:BS, 1:6], l5[s][t % 2][:],
                                ms[s][:], None, ALU.is_equal,
                            ).then_inc(s_dve)
                    if "tr" not in _SKIP:
                        for s in range(NS):
                            nc.vector.transpose(
                                mov[s][(t + 1) % 2][0:32, :], ohs[s][:],
                            ).then_inc(s_dve)

    return nc


def _prep(x, W_ih, W_hh, b_ih, b_hh, W_fc, b_fc, emb, TT):
    x = np.asarray(x, dtype=np.float32)
    W_ih = np.asarray(W_ih, dtype=np.float32)
    W_hh = np.asarray(W_hh, dtype=np.float32)
    b = np.asarray(b_ih, dtype=np.float32) + np.asarray(b_hh, dtype=np.float32)
    W_fc = np.asarray(W_fc, dtype=np.float32)
    b_fc = np.asarray(b_fc, dtype=np.float32)
    emb = np.asarray(emb, dtype=np.float32)

    # PyTorch gate rows [i, f, g, o] -> [i, f, o, g]
    perm = np.concatenate([np.arange(0, 64), np.arange(96, 128),
                           np.arange(64, 96)])
    W_ih_p = W_ih[perm]
    W_hh_p = W_hh[perm]
    b_p = b[perm]

    W_x = W_ih_p[:, :IN]              # [128, 64]
    W_p = W_ih_p[:, IN:]              # [128, 64]
    Wpe = W_p @ emb.T                 # [128, 5]

    wstack = np.zeros((97, 128), np.float32)
    wstack[1:6] = Wpe.T
    wstack[64:96] = W_hh_p.T
    wxT = np.ascontiguousarray(W_x.T)                # [64, 128]
    ball = np.ascontiguousarray(b_p.reshape(128, 1))
    wfc = np.zeros((97, NCLS), np.float32)
    wfc[64:96] = W_fc.T
    wfc[96] = b_fc

    in_maps = []
    for ci in range(NCORES):
        m = {"wstack": wstack, "wfc": wfc, "wxT": wxT, "ball": ball}
        for s in range(NS):
            r0 = ci * BL + s * BS
            xs = x[r0:r0 + BS, :TT]                  # [BS, TT, 64]
            y = xs.transpose(2, 1, 0)                # [64, TT, BS]
            m[f"xT{s}"] = np.ascontiguousarray(y.reshape(IN, TT * BS))
        in_maps.append(m)
    return in_maps


def kernel(x, x_lengths, edge_list, W_ih, W_hh, b_ih, b_hh, W_fc, b_fc, emb):
    global LAST_EXEC_NS, LAST_RESULTS
    TT = _TT
    inputs = _prep(x, W_ih, W_hh, b_ih, b_hh, W_fc, b_fc, emb, TT)

    ncores = int(os.environ.get("KB_CORES", NCORES))
    nc = _build(TT)
    res = run_bass_kernel_spmd(
        nc, inputs[:ncores], core_ids=list(range(ncores)), trace=_TRACE,
    )
    LAST_EXEC_NS = res.exec_time_ns
    LAST_RESULTS = res

    outs = [res.results[i]["out"].reshape(BL, TT, NCLS)
            for i in range(len(res.results))]
    while len(outs) < NCORES:
        outs.append(np.zeros((BL, TT, NCLS), np.float32))
    logits = np.concatenate(outs, axis=0)            # [256, TT, 5]
    m = logits.max(axis=-1, keepdims=True)
    z = logits - m
    logp = z - np.log(np.exp(z).sum(axis=-1, keepdims=True))
    if TT < T:
        pad = np.zeros((B, T - TT, NCLS), dtype=np.float32)
        logp = np.concatenate([logp, pad], axis=1)
    return logp.astype(np.float32)



# revision 3
# speedup vs baseline: 1.0671x; 1.0671x over previous
"""v2-bf16 + ACT logit copy + pairwise-interleaved argmax tail
(max0,max1,eq0,eq1,tr0,tr1: every dependent DVE pair spaced by one op).

Contract: kernel(**inputs) takes FULL unsharded numpy inputs, returns FULL
output [256, 2048, 5] f32 (per-step log_softmax of the decoder).

Design (per core, 32 batch rows, T=2048 sequential steps):
  - 2 phase-shifted streams of 16 batch rows; the streams' serial chains
    overlap across engines.
  - x @ Wx.T precomputed by bulk matmuls (16 steps at a time) directly into
    PSUM; the per-step matmul accumulates Whh@h + Wpe@onehot on top
    (start=False into a column slice).
  - Gate order [i,f,o,g]; ACT ops per step: sigmoid(rows 0:96), tanh(g) with
    base-shifted output, tanh(c). Cell math on DVE with partition-base-aligned
    operand pairs (verifier requires equal input bases for TensorTensor).
  - Argmax feedback: fc matmul (h stationary) -> l[16,5] psum; DVE copy->ls,
    reduce_max, is_equal, 32x32 transpose into the moving tile's onehot rows.
    Stream-interleaved DVE tail provides the hazard distance (no drains).
  - mov tile rows: 0=ones(tr col0), 1-5=onehot, 6-63=0, 64-95=h, 96=ones(fc
    bias row). Gates contraction = rows 0:97, fc contraction = rows 64:97.
  - Raw logits DMA'd out; log_softmax on host.
"""

import os
import numpy as np
import ml_dtypes
from contextlib import ExitStack

import concourse.bass as bass
from concourse import mybir
from concourse.bass_utils import run_bass_kernel_spmd

B, T, IN, H, NCLS = 256, 2048, 64, 32, 5
NCORES = 8
BL = B // NCORES          # 32 batch rows per core
NS = 2                    # streams per core
BS = BL // NS             # 16 batch rows per stream

AF = mybir.ActivationFunctionType
ALU = mybir.AluOpType
AX = mybir.AxisListType
F32 = mybir.dt.float32
BF16 = mybir.dt.bfloat16

_TT = int(os.environ.get("KB_TT", T))
_SKIP = set(filter(None, os.environ.get("KB_SKIP", "").split(",")))
_TRACE = os.environ.get("KB_TRACE", "0") == "1"

LAST_EXEC_NS = None
LAST_RESULTS = None


def _sched(TT, XWC):
    """Dry-run the emission order, assigning 1-based semaphore ticks."""
    tick = {}
    cnt = {"pe": 0, "act": 0, "dve": 0}

    def put(eng, key):
        cnt[eng] += 1
        tick[(eng,) + key] = cnt[eng]

    for s in range(NS):
        for k in range(7):
            put("dve", ("init", s, k))
    for t in range(TT):
        for s in range(NS):
            if t % XWC == 0:
                put("pe", ("bulk", t, s))
        for s in range(NS):
            put("pe", ("gates", t, s))
        if "fc" not in _SKIP:
            for s in range(NS):
                put("pe", ("fc", t, s))
        for s in range(NS):
            put("act", ("sig", t, s))
            put("act", ("tang", t, s))
        for s in range(NS):
            put("act", ("tanc", t, s))
        if "fc" not in _SKIP:
            for s in range(NS):
                put("act", ("copy", t, s))
        for s in range(NS):
            put("dve", ("mul2", t, s))
            put("dve", ("mul1", t, s))
            put("dve", ("add", t, s))
        for s in range(NS):
            put("dve", ("h", t, s))
        if "fc" not in _SKIP:
            for s in range(NS):
                put("dve", ("max", t, s))
            for s in range(NS):
                put("dve", ("eq", t, s))
        if "fc" not in _SKIP and "tr" not in _SKIP:
            for s in range(NS):
                put("dve", ("tr", t, s))
    return tick


def _build(TT):
    XWC = min(16, TT)          # steps per bulk xw matmul
    XC = min(128, TT)          # steps per x DMA chunk
    OC = min(256, TT)          # steps per output DMA chunk
    assert TT % XWC == 0 and TT % XC == 0 and TT % OC == 0

    tk = _sched(TT, XWC)
    nc = bass.Bass()

    x_d = [nc.declare_dram_parameter(f"xT{s}", [IN, TT * BS], BF16,
                                     isOutput=False) for s in range(NS)]
    ws_d = nc.declare_dram_parameter("wstack", [97, 128], BF16, isOutput=False)
    wf_d = nc.declare_dram_parameter("wfc", [97, NCLS], BF16, isOutput=False)
    wx_d = nc.declare_dram_parameter("wxT", [IN, 128], BF16, isOutput=False)
    b_d = nc.declare_dram_parameter("ball", [128, 1], F32, isOutput=False)
    out_d = nc.declare_dram_parameter("out", [BL, TT * NCLS], F32,
                                      isOutput=True)

    with ExitStack() as ctx:
        def sb(name, shape, dt=F32):
            return ctx.enter_context(nc.sbuf_tensor(name, shape, dt))

        xr = [[sb(f"xr{s}_{p}", [IN, XC * BS], BF16) for p in range(2)]
              for s in range(NS)]
        mov = [[sb(f"mov{s}_{p}", [97, 32], BF16) for p in range(2)]
               for s in range(NS)]
        S = [sb(f"S{s}", [96, BS]) for s in range(NS)]
        Q = [sb(f"Q{s}", [32, BS]) for s in range(NS)]       # tanh(g)
        C = [sb(f"C{s}", [64, BS]) for s in range(NS)]       # c at rows 32:64
        P = [sb(f"P{s}", [32, BS]) for s in range(NS)]       # i*tanh(g)
        P2 = [sb(f"P2{s}", [32, BS]) for s in range(NS)]     # f*c
        tco = [sb(f"tco{s}", [96, BS]) for s in range(NS)]   # tanh(c) @64:96
        ohs = [sb(f"ohs{s}", [32, 32], BF16) for s in range(NS)]
        ms = [sb(f"ms{s}", [BS, 1]) for s in range(NS)]
        lsb = [sb(f"ls{s}", [BS, TT * NCLS]) for s in range(NS)]
        wstack = sb("wstack_sb", [97, 128], BF16)
        wfc = sb("wfc_sb", [97, NCLS], BF16)
        wxT = sb("wxT_sb", [IN, 128], BF16)
        ball = sb("ball_sb", [128, 1])

        xwg = [[ctx.enter_context(
            nc.psum_tensor(f"xwg{s}_{p}", [128, XWC * BS], F32))
            for p in range(2)] for s in range(NS)]
        l5 = [[ctx.enter_context(
            nc.psum_tensor(f"l5{s}_{p}", [BS, NCLS], F32))
            for p in range(2)] for s in range(NS)]

        s_dmax = [ctx.enter_context(nc.semaphore(f"s_dmax{s}"))
                  for s in range(NS)]
        s_dmaw = ctx.enter_context(nc.semaphore("s_dmaw"))
        s_dmao = ctx.enter_context(nc.semaphore("s_dmao"))
        s_pe = ctx.enter_context(nc.semaphore("s_pe"))
        s_act = ctx.enter_context(nc.semaphore("s_act"))
        s_dve = ctx.enter_context(nc.semaphore("s_dve"))

        DVE_INIT = 7 * NS

        with nc.Block() as block:

            @block.sync
            def _(sync):
                for dst, src in ((wstack, ws_d), (wfc, wf_d), (wxT, wx_d),
                                 (ball, b_d)):
                    sync.dma_start(dst[:], src[:]).then_inc(s_dmaw, 16)
                for k in range(TT // XC):
                    for s in range(NS):
                        if k >= 2:
                            tlast = (k - 1) * XC - XWC
                            sync.wait_ge(s_pe, tk[("pe", "bulk", tlast, s)])
                        sync.dma_start(
                            xr[s][k % 2][:],
                            x_d[s][:, k * XC * BS:(k + 1) * XC * BS],
                        ).then_inc(s_dmax[s], 16)
                for j in range(TT // OC if "fc" not in _SKIP else 0):
                    tlast = (j + 1) * OC - 1
                    for s in range(NS):
                        sync.wait_ge(s_act, tk[("act", "copy", tlast, s)])
                        sync.dma_start(
                            out_d[s * BS:(s + 1) * BS,
                                  j * OC * NCLS:(j + 1) * OC * NCLS],
                            lsb[s][:, j * OC * NCLS:(j + 1) * OC * NCLS],
                        ).then_inc(s_dmao, 16)
                if "fc" not in _SKIP:
                    sync.wait_ge(s_dmao, 16 * (TT // OC) * NS)

            @block.tensor
            def _(pe):
                pe.wait_ge(s_dmaw, 64)
                for t in range(TT):
                    par = (t // XWC) % 2
                    for s in range(NS):
                        if t % XWC == 0:
                            if t % XC == 0:
                                k = t // XC
                                pe.wait_ge(s_dmax[s], 16 * (k + 1))
                            blk = t // XWC
                            if blk >= 2:
                                tlast = (blk - 1) * XWC - 1
                                pe.wait_ge(
                                    s_act, tk[("act", "tang", tlast, s)])
                            off = (t % XC) * BS
                            nc.tensor.matmul(
                                xwg[s][par][:],
                                wxT[:],
                                xr[s][(t // XC) % 2][:, off:off + XWC * BS],
                                start=True, stop=False,
                                skip_group_check=True,
                            ).then_inc(s_pe)
                    for s in range(NS):
                        if t == 0:
                            pe.wait_ge(s_dve, DVE_INIT)
                        else:
                            lbl = ("tr" if "tr" not in _SKIP else
                                   ("eq" if "fc" not in _SKIP else "h"))
                            pe.wait_ge(s_dve, tk[("dve", lbl, t - 1, s)])
                        sl = slice((t % XWC) * BS, (t % XWC + 1) * BS)
                        nc.tensor.matmul(
                            xwg[s][par][:, sl],
                            wstack[:],
                            mov[s][t % 2][0:97, 0:BS],
                            start=False, stop=True,
                            skip_group_check=True,
                        ).then_inc(s_pe)
                    if "fc" not in _SKIP:
                        for s in range(NS):
                            pe.wait_ge(s_dve, tk[("dve", "h", t, s)])
                            nc.tensor.matmul(
                                l5[s][t % 2][:],
                                mov[s][(t + 1) % 2][64:97, 0:BS],
                                wfc[64:97, :],
                                start=True, stop=True,
                            ).then_inc(s_pe)

            @block.scalar
            def _(act):
                for t in range(TT):
                    par = (t // XWC) % 2
                    sl = slice((t % XWC) * BS, (t % XWC + 1) * BS)
                    for s in range(NS):
                        act.wait_ge(s_pe, tk[("pe", "gates", t, s)])
                        nc.scalar.activation(
                            S[s][:], xwg[s][par][0:96, sl], AF.Sigmoid,
                            bias=ball[0:96, :],
                        ).then_inc(s_act)
                        nc.scalar.activation(
                            Q[s][:], xwg[s][par][96:128, sl], AF.Tanh,
                            bias=ball[96:128, :],
                        ).then_inc(s_act)
                    for s in range(NS):
                        act.wait_ge(s_dve, tk[("dve", "add", t, s)])
                        nc.scalar.activation(
                            tco[s][64:96, :], C[s][32:64, :], AF.Tanh,
                        ).then_inc(s_act)
                    if "fc" not in _SKIP:
                        for s in range(NS):
                            act.wait_ge(s_pe, tk[("pe", "fc", t, s)])
                            nc.scalar.copy(
                                lsb[s][:, t * NCLS:(t + 1) * NCLS],
                                l5[s][t % 2][:],
                            ).then_inc(s_act)

            @block.vector
            def _(dve):
                for s in range(NS):
                    for p in range(2):
                        nc.vector.memset(mov[s][p][:], 0.0).then_inc(s_dve)
                        nc.vector.memset(mov[s][p][96:97, :],
                                         1.0).then_inc(s_dve)
                    nc.vector.memset(C[s][32:64, :], 0.0).then_inc(s_dve)
                    nc.vector.memset(ohs[s][:], 0.0).then_inc(s_dve)
                    nc.vector.memset(ohs[s][:, 0:1], 1.0).then_inc(s_dve)
                for t in range(TT):
                    for s in range(NS):
                        dve.wait_ge(s_act, tk[("act", "sig", t, s)])
                        nc.vector.tensor_mul(
                            P2[s][:], S[s][32:64, :], C[s][32:64, :],
                        ).then_inc(s_dve)
                        dve.wait_ge(s_act, tk[("act", "tang", t, s)])
                        nc.vector.tensor_mul(
                            P[s][:], S[s][0:32, :], Q[s][:],
                        ).then_inc(s_dve)
                        nc.vector.tensor_add(
                            C[s][32:64, :], P[s][:], P2[s][:],
                        ).then_inc(s_dve)
                    for s in range(NS):
                        dve.wait_ge(s_act, tk[("act", "tanc", t, s)])
                        nc.vector.tensor_mul(
                            mov[s][(t + 1) % 2][64:96, 0:BS], S[s][64:96, :],
                            tco[s][64:96, :],
                        ).then_inc(s_dve)
                    if "fc" not in _SKIP:
                        for s in range(NS):
                            dve.wait_ge(s_pe, tk[("pe", "fc", t, s)])
                            nc.vector.reduce_max(
                                ms[s][:], l5[s][t % 2][:], axis=AX.X,
                            ).then_inc(s_dve)
                        for s in range(NS):
                            nc.vector.tensor_scalar(
                                ohs[s][0:BS, 1:6], l5[s][t % 2][:],
                                ms[s][:], None, ALU.is_equal,
                            ).then_inc(s_dve)
                        if "tr" not in _SKIP:
                            for s in range(NS):
                                nc.vector.transpose(
                                    mov[s][(t + 1) % 2][0:32, :], ohs[s][:],
                                ).then_inc(s_dve)

    return nc


def _prep(x, W_ih, W_hh, b_ih, b_hh, W_fc, b_fc, emb, TT):
    x = np.asarray(x, dtype=np.float32)
    W_ih = np.asarray(W_ih, dtype=np.float32)
    W_hh = np.asarray(W_hh, dtype=np.float32)
    b = np.asarray(b_ih, dtype=np.float32) + np.asarray(b_hh, dtype=np.float32)
    W_fc = np.asarray(W_fc, dtype=np.float32)
    b_fc = np.asarray(b_fc, dtype=np.float32)
    emb = np.asarray(emb, dtype=np.float32)

    # PyTorch gate rows [i, f, g, o] -> [i, f, o, g]
    perm = np.concatenate([np.arange(0, 64), np.arange(96, 128),
                           np.arange(64, 96)])
    W_ih_p = W_ih[perm]
    W_hh_p = W_hh[perm]
    b_p = b[perm]

    W_x = W_ih_p[:, :IN]              # [128, 64]
    W_p = W_ih_p[:, IN:]              # [128, 64]
    Wpe = W_p @ emb.T                 # [128, 5]

    wstack = np.zeros((97, 128), np.float32)
    wstack[1:6] = Wpe.T
    wstack[64:96] = W_hh_p.T
    bfd = ml_dtypes.bfloat16
    wxT = np.ascontiguousarray(W_x.T).astype(bfd)
    ball = np.ascontiguousarray(b_p.reshape(128, 1))
    wfc = np.zeros((97, NCLS), np.float32)
    wfc[64:96] = W_fc.T
    wfc[96] = b_fc
    wfc = wfc.astype(bfd)

    in_maps = []
    for ci in range(NCORES):
        m = {"wstack": wstack.astype(bfd), "wfc": wfc, "wxT": wxT,
             "ball": ball}
        for s in range(NS):
            r0 = ci * BL + s * BS
            xs = x[r0:r0 + BS, :TT]                  # [BS, TT, 64]
            y = xs.transpose(2, 1, 0)                # [64, TT, BS]
            m[f"xT{s}"] = np.ascontiguousarray(y.reshape(IN, TT * BS)).astype(bfd)
        in_maps.append(m)
    return in_maps


def kernel(x, x_lengths, edge_list, W_ih, W_hh, b_ih, b_hh, W_fc, b_fc, emb):
    global LAST_EXEC_NS, LAST_RESULTS
    TT = _TT
    inputs = _prep(x, W_ih, W_hh, b_ih, b_hh, W_fc, b_fc, emb, TT)

    ncores = int(os.environ.get("KB_CORES", NCORES))
    nc = _build(TT)
    res = run_bass_kernel_spmd(
        nc, inputs[:ncores], core_ids=list(range(ncores)), trace=_TRACE,
    )
    LAST_EXEC_NS = res.exec_time_ns
    LAST_RESULTS = res

    outs = [res.results[i]["out"].reshape(BL, TT, NCLS)
            for i in range(len(res.results))]
    while len(outs) < NCORES:
        outs.append(np.zeros((BL, TT, NCLS), np.float32))
    logits = np.concatenate(outs, axis=0)            # [256, TT, 5]
    m = logits.max(axis=-1, keepdims=True)
    z = logits - m
    logp = z - np.log(np.exp(z).sum(axis=-1, keepdims=True))
    if TT < T:
        pad = np.zeros((B, T - TT, NCLS), dtype=np.float32)
        logp = np.concatenate([logp, pad], axis=1)
    return logp.astype(np.float32)

